# revision 4
# baseline (speedup 1.0000x reference)
"""Trainium2 Bass kernel for a GQA causal attention layer (Llama-style).

Problem: x[2, 2048, 4096], 32 q heads / 8 kv heads, head_dim 128,
interleaved RoPE, causal softmax, output projection.

Distribution: 8-way tensor parallelism over heads. Each NeuronCore gets
4 q heads and 1 kv head (wq/wk/wv sharded along their out dim, wo along
its in dim). Per-core attention-output slices are exchanged via
AllGather (bf16, split per 512-token slice), after which each core
computes a 512-wide slice of the output projection; the full output is
reassembled on the host.

Schedule (v2): two dense PE segments.
  segment A: Q^T/K^T/V^T projections from x^T (f32r matmuls), RoPE via
    swap-matmul + DVE ops. K^T and V stay resident in SBUF; Q^T round-
    trips through DRAM (gpsimd queue). All PSUM->SBUF copies are on the
    vector engine so the scalar engine is free.
  segment B: causal attention (S^T = K @ Q^T orientation) fused with
    the output projection. The softmax denominator is accumulated on
    the vector engine (DVE) instead of a ones-matmul, cutting PE work
    by a third; exp runs on the scalar engine; the PE stream for each
    (head, q-chunk, kt) is [scores(kt), out-proj filler matmuls,
    PV(kt)] so the PE never waits for exp. Out-projection "filler"
    units (one attnF tile load + 4 matmuls) are interleaved one
    exchange-slice behind the attention stream, hiding the AllGather
    latency and the exp dependency entirely.
  dtypes: x/wq/wk/wv/q/k f32r (full PE rate, ~1e-4 err); V, P=exp(S),
    exchanged attention output and wo in bf16 (halves exchange and
    phase-3 DMA traffic; error stays ~1e-3).
"""

import numpy as np
import ml_dtypes

import concourse.bass as bass
import concourse.mybir as mybir
import concourse.tile as tile
from concourse import bacc
from concourse.masks import make_identity

F32 = mybir.dt.float32
F32R = mybir.dt.float32r
BF16 = mybir.dt.bfloat16
AF = mybir.ActivationFunctionType

N_CORES = 8
DIM = 4096
SEQ = 2048
BATCH = 2
N_HEADS = 32
N_KV_HEADS = 8
HEAD_DIM = 128
H_LOC = N_HEADS // N_CORES          # 4 q heads per core
E_LOC = H_LOC * HEAD_DIM            # 512
TOK = BATCH * SEQ                   # 4096
N_KT = DIM // 128                   # 32 contraction tiles for projections
N_CHUNK = TOK // 512                # 8 token chunks / exchange slices
SCALE = 1.0 / float(np.sqrt(HEAD_DIM))
PF_DEPTH = 3                        # attnF tile prefetch skew (filler units)


def _build():
    nc = bacc.Bacc("TRN2", target_bir_lowering=False, debug=False)

    xT = nc.declare_dram_parameter("xT", [DIM, TOK], F32R, isOutput=False)
    wqT = nc.declare_dram_parameter("wqT", [DIM, E_LOC], F32R, isOutput=False)
    wkT = nc.declare_dram_parameter("wkT", [DIM, HEAD_DIM], F32R, isOutput=False)
    wvT = nc.declare_dram_parameter("wvT", [DIM, HEAD_DIM], F32R, isOutput=False)
    woT = nc.declare_dram_parameter("woT", [DIM, E_LOC], BF16, isOutput=False)
    cos2 = nc.declare_dram_parameter("cos2", [128, SEQ], F32R, isOutput=False)
    sgnsin2 = nc.declare_dram_parameter("sgnsin2", [128, SEQ], F32R, isOutput=False)
    swp = nc.declare_dram_parameter("swp", [128, 128], F32R, isOutput=False)
    trimask = nc.declare_dram_parameter("trimask", [128, 128], BF16, isOutput=False)
    ones = nc.declare_dram_parameter("ones", [128, 128], F32R, isOutput=False)
    out = nc.declare_dram_parameter("out", [TOK, E_LOC], F32, isOutput=True)

    with tile.TileContext(nc) as tc:
        with tc.tile_pool(name="dram", bufs=1, space="DRAM") as dram:
            qT_d = dram.tile([E_LOC, TOK], F32R)
            attnL = [dram.tile([E_LOC, 512], BF16, name=f"attnL{m}")
                     for m in range(N_CHUNK)]
            attnF = [dram.tile([N_CORES * E_LOC, 512], BF16, addr_space="Shared",
                               name=f"attnF{m}")
                     for m in range(N_CHUNK)]

            # ---- constants + K/V residents (live for the whole kernel) ----
            with (
                tc.tile_pool(name="consts", bufs=1) as consts,
                tc.tile_pool(name="kv", bufs=1) as kv,
            ):
                swp_sb = consts.tile([128, 128], F32R)
                nc.sync.dma_start(out=swp_sb, in_=swp[:])
                trimask_sb = consts.tile([128, 128], BF16)
                nc.sync.dma_start(out=trimask_sb, in_=trimask[:])
                ones_sb = consts.tile([128, 128], F32R)
                nc.sync.dma_start(out=ones_sb, in_=ones[:])
                cos2_sb = consts.tile([128, SEQ], F32R)
                nc.sync.dma_start(out=cos2_sb, in_=cos2[:])
                sgnsin2_sb = consts.tile([128, SEQ], F32R)
                nc.sync.dma_start(out=sgnsin2_sb, in_=sgnsin2[:])
                ident_sb = consts.tile([128, 128], F32)
                make_identity(nc, ident_sb)

                kT_sb = kv.tile([128, TOK], F32R)          # K^T, both batches
                v3_sb = kv.tile([128, TOK // 128, 128], BF16)  # V, both batches

                # ================= segment A: projections + RoPE =============
                with (
                    tc.tile_pool(name="p1w", bufs=1) as p1w,
                    tc.tile_pool(name="p1x", bufs=10) as p1x,
                    tc.tile_pool(name="p1r", bufs=3) as p1r,
                    tc.tile_pool(name="p1acc", bufs=1, space="PSUM") as p1acc,
                    tc.tile_pool(name="p1aux", bufs=2, space="PSUM") as p1aux,
                ):
                    wq_sb = [None] * N_KT
                    wk_sb = [None] * N_KT
                    wv_sb = [None] * N_KT

                    for c in range(N_CHUNK):
                      with nc.named_scope(f"pA_c{c}"):
                        t0 = 512 * c
                        s0 = t0 % SEQ
                        ps_q = [p1acc.tile([128, 512], F32, name=f"psq{h}_{c}", tag=f"accq{h}")
                                for h in range(H_LOC)]
                        ps_k = p1acc.tile([128, 512], F32, name=f"psk_{c}", tag="acck")
                        ps_v = p1acc.tile([128, 512], F32, name=f"psv_{c}", tag="accv")
                        for kt in range(N_KT):
                            if c == 0:
                                # load weights on first use so chunk 0 can start
                                # after only a few DMAs (scalar HWDGE queue, in
                                # parallel with the x stream on sync)
                                wq_sb[kt] = p1w.tile([128, E_LOC], F32R, name=f"wq{kt}")
                                nc.scalar.dma_start(
                                    out=wq_sb[kt], in_=wqT[128 * kt:128 * (kt + 1), :])
                                wk_sb[kt] = p1w.tile([128, HEAD_DIM], F32R, name=f"wk{kt}")
                                nc.scalar.dma_start(
                                    out=wk_sb[kt], in_=wkT[128 * kt:128 * (kt + 1), :])
                                wv_sb[kt] = p1w.tile([128, HEAD_DIM], F32R, name=f"wv{kt}")
                                nc.scalar.dma_start(
                                    out=wv_sb[kt], in_=wvT[128 * kt:128 * (kt + 1), :])
                            xt = p1x.tile([128, 512], F32R, name=f"xt_{c}_{kt}", tag="xt")
                            nc.sync.dma_start(
                                out=xt, in_=xT[128 * kt:128 * (kt + 1), t0:t0 + 512])
                            st = kt == 0
                            sp = kt == N_KT - 1
                            for h in range(H_LOC):
                                nc.tensor.matmul(
                                    ps_q[h][:], wq_sb[kt][:, 128 * h:128 * (h + 1)],
                                    xt[:], start=st, stop=sp)
                            nc.tensor.matmul(ps_k[:], wk_sb[kt][:], xt[:], start=st, stop=sp)
                            nc.tensor.matmul(ps_v[:], wv_sb[kt][:], xt[:], start=st, stop=sp)

                        # RoPE: k first (phase B needs kT earliest), then q heads.
                        # All PSUM->SBUF copies on DVE; the swap matmul on PE.
                        rope_jobs = [(ps_k, None, 0)]
                        rope_jobs += [(ps_q[h], qT_d, 128 * h) for h in range(H_LOC)]
                        for j, (ps, dst, row) in enumerate(rope_jobs):
                            t_sb = p1r.tile([128, 512], F32R, name=f"t1_{c}_{j}", tag="t1")
                            nc.vector.tensor_copy(t_sb[:], ps[:])
                            ps2 = p1aux.tile([128, 512], F32, name=f"ps2_{c}_{j}", tag="aux")
                            nc.tensor.matmul(ps2[:], swp_sb[:], t_sb[:], start=True, stop=True)
                            m1 = p1r.tile([128, 512], F32R, name=f"m1_{c}_{j}", tag="m1")
                            nc.vector.tensor_mul(m1[:], t_sb[:], cos2_sb[:, s0:s0 + 512])
                            ro = p1r.tile([128, 512], F32R, name=f"ro_{c}_{j}", tag="ro")
                            nc.vector.tensor_mul(ro[:], ps2[:], sgnsin2_sb[:, s0:s0 + 512])
                            if dst is None:
                                # k: write straight into the SBUF resident
                                nc.vector.tensor_add(
                                    kT_sb[:, t0:t0 + 512], ro[:], m1[:])
                            else:
                                nc.vector.tensor_add(ro[:], ro[:], m1[:])
                                nc.gpsimd.dma_start(
                                    out=dst[row:row + 128, t0:t0 + 512], in_=ro[:])

                        # V: transpose V^T chunk [128 e, 512 tok] -> V [tok, e]
                        v_sb = p1r.tile([128, 512], F32, name=f"vsb_{c}", tag="vsb")
                        nc.vector.tensor_copy(v_sb[:], ps_v[:])
                        for j in range(4):
                            pt = p1aux.tile([128, 128], F32, name=f"pvt_{c}_{j}", tag="aux")
                            nc.tensor.transpose(pt[:], v_sb[:, 128 * j:128 * (j + 1)], ident_sb[:])
                            nc.vector.tensor_copy(v3_sb[:, 4 * c + j, :], pt[:])

                # ========== segment B: attention fused with out-proj =========
                with (
                    tc.tile_pool(name="p3w", bufs=1) as p3w,
                    tc.tile_pool(name="p2q", bufs=8) as p2q,
                    tc.tile_pool(name="p2p", bufs=6) as p2p,
                    tc.tile_pool(name="p2den", bufs=2) as p2den,
                    tc.tile_pool(name="p2o", bufs=3) as p2o,
                    tc.tile_pool(name="p3a", bufs=8) as p3a,
                    tc.tile_pool(name="p3o", bufs=3) as p3o,
                    tc.tile_pool(name="psS", bufs=2, space="PSUM") as psS,
                    tc.tile_pool(name="psO", bufs=2, space="PSUM") as psO,
                    tc.tile_pool(name="psF", bufs=1, space="PSUM") as psF,
                ):
                    wo_sb = []
                    for kt in range(N_KT):
                        wo_t = p3w.tile([128, E_LOC], BF16, name=f"wo{kt}")
                        nc.scalar.dma_start(out=wo_t, in_=woT[128 * kt:128 * (kt + 1), :])
                        wo_sb.append(wo_t)

                    # ---- out-projection filler machinery ----
                    p3_queue = []          # FIFO of emit-callables
                    p3_state = {}          # mt -> {'a': [...], 'pf': [...]}

                    def p3_unit(mt, j):
                        def emit():
                            st = p3_state.setdefault(mt, {'a': [None] * N_KT, 'pf': None})
                            if j == 0:
                                st['pf'] = [psF.tile([128, 512], F32,
                                                     name=f"pf_{mt}_{s}", tag=f"o3_{s}")
                                            for s in range(4)]
                                for jj in range(PF_DEPTH):
                                    st['a'][jj] = p3a.tile([128, 512], BF16,
                                                           name=f"a_{mt}_{jj}", tag="att")
                                    nc.sync.dma_start(
                                        out=st['a'][jj],
                                        in_=attnF[mt][128 * jj:128 * (jj + 1), :])
                            jj = j + PF_DEPTH
                            if jj < N_KT:
                                st['a'][jj] = p3a.tile([128, 512], BF16,
                                                       name=f"a_{mt}_{jj}", tag="att")
                                nc.sync.dma_start(
                                    out=st['a'][jj],
                                    in_=attnF[mt][128 * jj:128 * (jj + 1), :])
                            a_sb = st['a'][j]
                            for s in range(4):
                                nc.tensor.matmul(
                                    st['pf'][s][:], a_sb[:, 128 * s:128 * (s + 1)],
                                    wo_sb[j][:], start=(j == 0), stop=(j == N_KT - 1))
                            st['a'][j] = None
                        return emit

                    def p3_end(mt):
                        def emit():
                            st = p3_state[mt]
                            for s in range(4):
                                o_sb = p3o.tile([128, 512], F32, name=f"ob_{mt}_{s}", tag="os")
                                nc.vector.tensor_copy(o_sb[:], st['pf'][s][:])
                                nc.sync.dma_start(
                                    out=out[512 * mt + 128 * s:512 * mt + 128 * (s + 1), :],
                                    in_=o_sb[:])
                        return emit

                    def enqueue_slice(mt):
                        for j in range(N_KT):
                            p3_queue.append(p3_unit(mt, j))
                        p3_queue.append(p3_end(mt))

                    def drain(n):
                        for _ in range(n):
                            if not p3_queue:
                                return
                            p3_queue.pop(0)()

                    # ---- attention stream ----
                    for b in range(BATCH):
                        tb = SEQ * b
                        kt_tiles = [kT_sb[:, tb + 128 * j:tb + 128 * (j + 1)]
                                    for j in range(SEQ // 128)]
                        v_tiles = [v3_sb[:, (SEQ // 128) * b + j, :]
                                   for j in range(SEQ // 128)]
                        qT_sb = [p2q.tile([128, SEQ], F32R, name=f"q_{b}_{h}", tag="qT")
                                 for h in range(H_LOC)]
                        for cc in range(SEQ // 512):
                            t1_ = tb + 512 * cc
                            for h in range(H_LOC):
                                nc.gpsimd.dma_start(
                                    out=qT_sb[h][:, 512 * cc:512 * (cc + 1)],
                                    in_=qT_d[128 * h:128 * (h + 1), t1_:t1_ + 512])
                        for c2 in range(SEQ // 512):
                            n_kt = 4 * c2 + 4
                            m = 4 * b + c2
                            with nc.named_scope(f"pB_b{b}c{c2}"):
                                for h in range(H_LOC):
                                    ps_o = psO.tile([128, 512], F32,
                                                    name=f"o_{b}_{h}_{c2}", tag="oT")
                                    den = p2den.tile([128, 512], F32R,
                                                     name=f"dn_{b}_{h}_{c2}", tag="den")
                                    for kt in range(n_kt):
                                        col_lo = max(0, 128 * kt - 512 * c2)
                                        width = 512 - col_lo
                                        ps_s = psS.tile([128, 512], F32,
                                                        name=f"s_{b}_{h}_{c2}_{kt}", tag="sT")
                                        nc.tensor.matmul(
                                            ps_s[:, 0:width],
                                            kt_tiles[kt][:],
                                            qT_sb[h][:, 512 * c2 + col_lo:512 * (c2 + 1)],
                                            start=True, stop=True)
                                        pT = p2p.tile([128, 512], BF16,
                                                      name=f"p_{b}_{h}_{c2}_{kt}", tag="pT")
                                        nc.scalar.activation(
                                            pT[:, 0:width], ps_s[:, 0:width],
                                            AF.Exp, scale=SCALE)
                                        if kt >= 4 * c2:
                                            nc.vector.tensor_mul(
                                                pT[:, 0:128], pT[:, 0:128], trimask_sb[:])
                                        # denominator accumulates on DVE
                                        if kt == 0:
                                            nc.vector.tensor_copy(den[:], pT[:])
                                        else:
                                            nc.vector.tensor_add(
                                                den[:, col_lo:512], den[:, col_lo:512],
                                                pT[:, 0:width])
                                        # filler keeps the PE busy while exp runs
                                        drain(1)
                                        nc.tensor.matmul(
                                            ps_o[:, col_lo:512], v_tiles[kt][:],
                                            pT[:, 0:width],
                                            start=(kt == 0), stop=(kt == n_kt - 1))
                                    # denominator partition-reduce + normalize
                                    ps_d = psS.tile([128, 512], F32,
                                                    name=f"d_{b}_{h}_{c2}", tag="sT")
                                    nc.tensor.matmul(ps_d[:], ones_sb[:], den[:],
                                                     start=True, stop=True)
                                    rec = p2o.tile([128, 512], F32,
                                                   name=f"r_{b}_{h}_{c2}", tag="rec")
                                    nc.vector.reciprocal(rec[:], ps_d[:])
                                    oT = p2o.tile([128, 512], BF16,
                                                  name=f"ot_{b}_{h}_{c2}", tag="oTs")
                                    nc.vector.tensor_mul(oT[:], ps_o[:], rec[:])
                                    nc.gpsimd.dma_start(
                                        out=attnL[m][128 * h:128 * (h + 1), :], in_=oT[:])
                                # token slice m complete on this core -> exchange
                                nc.gpsimd.collective_compute(
                                    "AllGather",
                                    mybir.AluOpType.bypass,
                                    replica_groups=[list(range(N_CORES))],
                                    ins=[attnL[m].opt()],
                                    outs=[attnF[m].opt()],
                                )
                            if m >= 1:
                                # out-proj for slice m-1 becomes eligible one
                                # slice behind the attention stream
                                enqueue_slice(m - 1)

                    enqueue_slice(N_CHUNK - 1)
                    drain(len(p3_queue))

    nc.compile()
    return nc


def _host_inputs(x, freqs_cos, freqs_sin, wq, wk, wv, wo):
    """Build the per-core input maps from the full problem inputs."""
    x = np.asarray(x, dtype=np.float32)
    freqs_cos = np.asarray(freqs_cos, dtype=np.float32)
    freqs_sin = np.asarray(freqs_sin, dtype=np.float32)
    wq = np.asarray(wq, dtype=np.float32)
    wk = np.asarray(wk, dtype=np.float32)
    wv = np.asarray(wv, dtype=np.float32)
    wo = np.asarray(wo, dtype=np.float32)

    xT = np.ascontiguousarray(x.reshape(TOK, DIM).T)

    # RoPE helper tiles: row r pairs with freq r//2
    cos2 = np.empty((128, SEQ), np.float32)
    sgnsin2 = np.empty((128, SEQ), np.float32)
    cT = freqs_cos.T  # [64, SEQ]
    sT = freqs_sin.T
    cos2[0::2, :] = cT
    cos2[1::2, :] = cT
    sgnsin2[0::2, :] = -sT
    sgnsin2[1::2, :] = sT

    swp = np.zeros((128, 128), np.float32)
    for j in range(64):
        swp[2 * j, 2 * j + 1] = 1.0
        swp[2 * j + 1, 2 * j] = 1.0

    trimask = np.triu(np.ones((128, 128), np.float32)).astype(ml_dtypes.bfloat16)
    ones = np.ones((128, 128), np.float32)
    woT = np.ascontiguousarray(wo.T).astype(ml_dtypes.bfloat16)  # [E, D]

    in_maps = []
    for i in range(N_CORES):
        in_maps.append({
            "xT": xT,
            "wqT": np.ascontiguousarray(wq[E_LOC * i:E_LOC * (i + 1), :].T),
            "wkT": np.ascontiguousarray(wk[HEAD_DIM * i:HEAD_DIM * (i + 1), :].T),
            "wvT": np.ascontiguousarray(wv[HEAD_DIM * i:HEAD_DIM * (i + 1), :].T),
            "woT": np.ascontiguousarray(woT[:, E_LOC * i:E_LOC * (i + 1)]),
            "cos2": cos2,
            "sgnsin2": sgnsin2,
            "swp": swp,
            "trimask": trimask,
            "ones": ones,
        })
    return in_maps


def _assemble(results):
    """Concatenate per-core output slices into the full [B, S, D] output."""
    full = np.concatenate([results[i]["out"] for i in range(N_CORES)], axis=1)
    return full.reshape(BATCH, SEQ, DIM)


_NC_CACHE = None


def _get_nc():
    global _NC_CACHE
    if _NC_CACHE is None:
        _NC_CACHE = _build()
    return _NC_CACHE


def run(inputs, trace=False):
    """Run the SPMD kernel on cores 0-7; returns (full_output, results)."""
    from concourse.bass_utils import run_bass_kernel_spmd
    nc = _get_nc()
    in_maps = _host_inputs(**inputs)
    res = run_bass_kernel_spmd(nc, in_maps, list(range(N_CORES)), trace=trace)
    return _assemble(res.results), res


def kernel(x, freqs_cos, freqs_sin, wq, wk, wv, wo):
    out, _ = run(dict(x=x, freqs_cos=freqs_cos, freqs_sin=freqs_sin,
                      wq=wq, wk=wk, wv=wv, wo=wo))
    return out
